# revision 23
# baseline (speedup 1.0000x reference)
"""Trainium2 Bass kernel for nn_BinarySurrogateBlock.

Computes y = x @ W^T where W = (sum_k 2^bits[k] * (pos_k - neg_k)) / scale.

Primary mode "hybrid" (ladder bits, the spec's arange fill): 8 cores =
2 token-halves x 4 out-quarters; per core 8192 tokens x 1024 outs x full
4096 contraction. The bit planes pack into one byte per weight per sign
(packbits); W = wp - wn is dequantized once on the DVE (exact in bf16,
|W_int| <= 255) and kept SBUF-resident while x streams through as the
stationary matmul operand (bf16 x bf16 -> fp32 PSUM, N=512 moving). The
4096 matmuls/core run back-to-back at the 512-cycle PE floor; startup
streams masks + first x groups in ic-granular, ring-balanced chunks so
the PE crawls with arrival instead of stalling (the first ~90 us are
chip-HBM-bound), and y is written as fp16 (halved write traffic; error
is far below the bf16 input rounding that dominates).

Note: 8-bit PE modes cannot beat this. FP8 DoubleRow measures the same
216 ns per N=512 matmul as bf16 (2x MACs per matmul via 256-contraction,
so 157 TF/s peak), but exact W in [-255, 255] needs two e4m3 planes
(16*Whi + Wlo), which exactly cancels the 2x. Single-plane fp8 x fails
the 2e-2 gate (measured rel err 2.6e-2); uint8 matmul is rejected by
bass.

Fallback modes for non-ladder bits: "pe" (fp8 DoubleRow dequant against
+/-2^bits patterns) and "dve" (vector-engine plane accumulation), both
feeding the original tensor-parallel-over-d_out main loop.
"""

import numpy as np
import ml_dtypes

# Problem shape (hardcoded per contract; kernel.py must be self-contained).
B, T, D_IN, D_OUT, K = 8, 2048, 4096, 4096, 8
N_CORES = 8
TOK = B * T                    # 16384 tokens
O_PER = D_OUT // N_CORES       # 512 outputs per core
P = 128                        # partitions
IC = D_IN // P                 # 32 contraction chunks
TSUP = 512                     # token super-tile width
NSUP = TOK // TSUP             # 32 super-tiles
TS_PER = TSUP // P             # 4 psum tiles per super-tile
IB = 16                        # i-rows dequantized per PE-dequant matmul
NB = D_IN // IB                # 256 dequant blocks
BG = 4                         # blocks per mask DMA
DEQUANT_MODE = "pe"

LAST_RESULTS = None            # BassKernelResults of the last run (for test.py)

_CACHE = {}

# ---- Hybrid-sharded bf16 kernel (primary path) ----
# 8 cores = 2 token-halves x 4 out-quarters. Per core: 8192 tokens x 1024
# outs x full 4096 contraction. W = packbits(pos) - packbits(neg) (int,
# |W|<=255, exact in bf16) is dequantized once on the DVE and kept
# SBUF-resident; x streams through as the stationary matmul operand.
# Chip-wide HBM traffic drops from ~1.3 GB (TP-8) to ~0.83 GB, removing
# the DMA-contention stall mode, while the PE stays at the same
# 4096-matmul cycle floor.
G_TOK = 2                      # token-parallel ways
H_OUT = 4                      # out-parallel ways
T_C = TOK // G_TOK             # 8192 tokens per core
O_C = D_OUT // H_OUT           # 1024 outs per core
SW = 512                       # tokens per sweep (4 psum tchunks x 2 oc = 8 banks)
NSW = T_C // SW                # 16 sweeps
OC2 = O_C // 512               # 2 psum-width output chunks


def _build_program_hybrid(c0_scale):
    import concourse.mybir as mybir
    import concourse.tile as tile
    from concourse import bacc
    from concourse.tile_rust import add_dep_helper

    dt = mybir.dt
    nc = bacc.Bacc("TRN2", target_bir_lowering=False, debug=False)

    TG = 1024                  # tokens per group (8 psum banks x 128)
    NG = T_C // TG             # 8 groups per core

    xt = nc.dram_tensor("xt", [D_IN, T_C], dt.bfloat16, kind="ExternalInput")
    wm = nc.dram_tensor("wm", [IC, 2, P, 2, 512], dt.uint8,
                        kind="ExternalInput")
    y = nc.dram_tensor("y", [T_C, O_C], dt.float16, kind="ExternalOutput")
    nc.tensors = {"xt": xt, "y": y}

    with tile.TileContext(nc) as tc:
        with (
            tc.tile_pool(name="wres", bufs=1) as wres,
            tc.tile_pool(name="mpool", bufs=6) as mpool,
            tc.tile_pool(name="xpool", bufs=2) as xpool,
            tc.tile_pool(name="ypool", bufs=2) as ypool,
            tc.tile_pool(name="psum", bufs=8, space="PSUM") as psum,
        ):
            xt_v = xt.rearrange("(ic p) t -> p ic t", p=P)   # [128, IC, T_C]
            y_vp = y.rearrange("(n p) o -> p n o", p=P)      # [128, T_C//P, O_C]
            w = wres.tile([P, IC, O_C], dt.bfloat16)

            # Per-HWDGE-ring FIFO order: chain each DMA behind the previous
            # one on its ring so scheduler priorities can't reorder the
            # startup mask stream vs later prefetches.
            last_dma = {"sync": None, "act": None}

            def ring_dma(ring, dst, src):
                eng = nc.sync if ring == "sync" else nc.scalar
                dma = eng.dma_start(dst, src)
                if last_dma[ring] is not None:
                    add_dep_helper(dma.ins, last_dma[ring].ins, sync=False,
                                   reason="ring FIFO order")
                last_dma[ring] = dma
                return dma

            # Drain 8 psum banks of one sweep (g, oc): scale-copy each bank
            # to fp16 (alternating scalar/vector engines so bunched closes
            # pipeline), then write y as two 4-bank DMAs on the sync ring.
            def drain_sweep(banks, g, oc, fine=False):
                for half in range(2):
                    yt = ypool.tile([P, 4, 512], dt.float16, tag="yt")
                    for q in range(4):
                        b = half * 4 + q
                        if b % 2 == 0:
                            nc.scalar.activation(
                                yt[:, q, :], banks[b][:],
                                mybir.ActivationFunctionType.Copy,
                                scale=float(c0_scale))
                        else:
                            nc.vector.tensor_scalar_mul(
                                yt[:, q, :], banks[b][:], float(c0_scale))
                        if fine:
                            # Tail sweep: ship each bank as soon as its
                            # scale-copy lands so the final transfer is
                            # 128 KB, not 512 KB, and rides both rings.
                            n = g * 8 + b
                            ring_dma("sync" if b % 2 == 0 else "act",
                                     y_vp[:, n:n + 1,
                                          oc * 512:(oc + 1) * 512],
                                     yt[:, q:q + 1, :])
                    if not fine:
                        n0 = g * 8 + half * 4
                        ring_dma("sync",
                                 y_vp[:, n0:n0 + 4, oc * 512:(oc + 1) * 512],
                                 yt[:])

            def mm_sweep_ic(banks, xg, ic, oc, first, last):
                for tcn in range(8):
                    nc.tensor.matmul(
                        banks[tcn],
                        xg[:, ic, tcn * P:(tcn + 1) * P],
                        w[:, ic, oc * 512:(oc + 1) * 512],
                        start=first, stop=last)

            psA = [psum.tile([P, 512], dt.float32, name=f"psA_{b}", tag="ps")
                   for b in range(8)]

            # Sweep A: group 0, oc 0, ic-major. Per ic: lo-half masks on the
            # act ring, x split 3:1 sync:act, 8 matmuls. Startup is
            # chip-HBM-bound, so demand is balanced across rings and kept
            # fine-grained (sub-us stalls keep the HAM gate open).
            xg0 = xpool.tile([P, IC, TG], dt.bfloat16, tag="xs")
            b_pre = {}
            for ic in range(IC):
                wpn = mpool.tile([P, 2, 512], dt.uint8, tag="wpn")
                ring_dma("act" if ic % 2 == 0 else "sync", wpn[:], wm[ic, 0])
                nc.vector.tensor_tensor(w[:, ic, 0:512], wpn[:, 0, :],
                                        wpn[:, 1, :], mybir.AluOpType.subtract)
                ring_dma("sync" if ic % 2 == 0 else "act",
                         xg0[:, ic, :], xt_v[:, ic, 0:TG])
                mm_sweep_ic(psA, xg0, ic, 0, ic == 0, ic == IC - 1)
                if ic >= IC - 8 and (ic - (IC - 8)) % 2 == 0:
                    # Pre-stage sweep B's first mask chunks so B starts
                    # without waiting for the ring to drain A's stream.
                    j = (ic - (IC - 8)) // 2
                    wpn_b = mpool.tile([P, 2, 512], dt.uint8, tag="wpn")
                    ring_dma("act" if j % 2 == 0 else "sync",
                             wpn_b[:], wm[j, 1])
                    b_pre[j] = wpn_b
            drain_sweep(psA, 0, 0)

            # Sweep B: group 0, oc 1, ic-major. Hi-half masks on the act
            # ring; x is already resident. Group 1's x prefetches into the
            # spare sync bandwidth.
            xg1 = xpool.tile([P, IC, TG], dt.bfloat16, tag="xs")
            psB = [psum.tile([P, 512], dt.float32, name=f"psB_{b}", tag="ps")
                   for b in range(8)]
            for ic in range(IC):
                if ic in b_pre:
                    wpn = b_pre.pop(ic)
                else:
                    wpn = mpool.tile([P, 2, 512], dt.uint8, tag="wpn")
                    ring_dma("act" if ic % 2 == 0 else "sync",
                             wpn[:], wm[ic, 1])
                nc.vector.tensor_tensor(w[:, ic, 512:O_C], wpn[:, 0, :],
                                        wpn[:, 1, :], mybir.AluOpType.subtract)
                ring_dma("sync" if ic % 2 == 0 else "act",
                         xg1[:, ic, 0:512], xt_v[:, ic, TG:TG + 512])
                mm_sweep_ic(psB, xg0, ic, 1, ic == 0, ic == IC - 1)
            drain_sweep(psB, 0, 1)

            # Steady state: for each group, oc 0 then oc 1, tc-major so psum
            # banks close staggered. The next group's x rides the act ring
            # (idle after the masks), so y drains on sync are never blocked
            # behind a bulk transfer.
            # Group 1, oc 0: still draining the tail of xg1's transfer, so
            # run ic-major to crawl with arrival instead of a flat stall.
            psC = [psum.tile([P, 512], dt.float32, name=f"psC_{b}", tag="ps")
                   for b in range(8)]
            for ic in range(IC):
                ring_dma("sync" if ic % 2 == 0 else "act",
                         xg1[:, ic, 512:TG], xt_v[:, ic, TG + 512:2 * TG])
            for ic in range(IC):
                mm_sweep_ic(psC, xg1, ic, 0, ic == 0, ic == IC - 1)
            drain_sweep(psC, 1, 0)

            xg_cur = xg1
            for g in range(1, NG):
                xg_nxt = None
                if g + 1 < NG:
                    xg_nxt = xpool.tile([P, IC, TG], dt.bfloat16, tag="xs")
                for oc in range(OC2):
                    if g == 1 and oc == 0:
                        if xg_nxt is not None:
                            ring_dma("act", xg_nxt[:, 0:IC // 2, :],
                                     xt_v[:, 0:IC // 2,
                                          (g + 1) * TG:(g + 2) * TG])
                        continue
                    if xg_nxt is not None:
                        ring_dma("act",
                                 xg_nxt[:, oc * (IC // 2):(oc + 1) * (IC // 2), :],
                                 xt_v[:, oc * (IC // 2):(oc + 1) * (IC // 2),
                                      (g + 1) * TG:(g + 2) * TG])
                    banks = []
                    for tcn in range(8):
                        ps = psum.tile([P, 512], dt.float32, tag="ps")
                        for ic in range(IC):
                            nc.tensor.matmul(
                                ps[:],
                                xg_cur[:, ic, tcn * P:(tcn + 1) * P],
                                w[:, ic, oc * 512:(oc + 1) * 512],
                                start=(ic == 0), stop=(ic == IC - 1))
                        banks.append(ps)
                    drain_sweep(banks, g, oc,
                                fine=(g == NG - 1 and oc == OC2 - 1))
                xg_cur = xg_nxt

    nc.compile()
    return nc


def _build_common(nc, mybir, tile, tc, pools, w, inv_scale, late_mask_dmas=None):
    """Main matmul phase: x-stationary, psum [128 tokens, 512 outs]."""
    from concourse.tile_rust import add_dep_helper
    dt = mybir.dt
    xpool, ypool, psum = pools
    xt = nc.tensors["xt"]
    y = nc.tensors["y"]
    xt_v = xt.rearrange("(ic p) t -> p ic t", p=P)     # [128, IC, TOK]
    y_v = y.rearrange("(n p) o -> n p o", p=P)         # [TOK//P, 128, O_PER]
    for st in range(NSUP):
        xt_t = xpool.tile([P, IC, TSUP], dt.bfloat16)
        # First super-tiles arrive in smaller pieces so the mains can start
        # as soon as the first token sub-tile lands (startup HBM congestion).
        npiece = 4 if st == 0 else (2 if st == 1 else 1)
        pw = TSUP // npiece
        for pc in range(npiece):
            x_dma = nc.sync.dma_start(
                xt_t[:, :, pc * pw:(pc + 1) * pw],
                xt_v[:, :, st * TSUP + pc * pw:st * TSUP + (pc + 1) * pw])
            if late_mask_dmas is not None and st < len(late_mask_dmas):
                # Keep the hoistable x prefetches from injecting into the
                # latency-critical mask stream on the same HWDGE FIFO ring.
                add_dep_helper(
                    x_dma.ins, late_mask_dmas[st].ins, sync=False,
                    reason="delay x prefetch behind dequant mask stream")
        for ts in range(TS_PER):
            ps = psum.tile([P, O_PER], dt.float32)
            for ic in range(IC):
                nc.tensor.matmul(
                    ps[:],
                    xt_t[:, ic, ts * P:(ts + 1) * P],
                    w[:, ic, :],
                    start=(ic == 0),
                    stop=(ic == IC - 1),
                )
            yt = ypool.tile([P, O_PER], dt.float32)
            nc.scalar.activation(
                yt[:], ps[:], mybir.ActivationFunctionType.Copy,
                scale=float(inv_scale))
            nc.scalar.dma_start(y_v[st * TS_PER + ts], yt[:])


def _build_program_pe(coeffs, inv_scale):
    import concourse.mybir as mybir
    import concourse.tile as tile
    from concourse import bacc

    dt = mybir.dt
    nc = bacc.Bacc("TRN2", target_bir_lowering=False, debug=False)
    nc.tensors = {}

    BPC = P // (2 * IB)  # dequant blocks (32 i-rows) per W chunk (4)

    xt = nc.dram_tensor("xt", [D_IN, TOK], dt.bfloat16, kind="ExternalInput")
    # DoubleRow rhs layout: [32-row block, ki=(k,i16), ko, o]
    NB32 = D_IN // (2 * IB)
    posm = nc.dram_tensor("posm", [NB32, P, 2, O_PER], dt.float8e4,
                          kind="ExternalInput")
    negm = nc.dram_tensor("negm", [NB32, P, 2, O_PER], dt.float8e4,
                          kind="ExternalInput")
    # lconst[s, j, ki, ko, p]: +/- 2^bits patterns; group j places dequant
    # block j at output partitions [j*32, (j+1)*32); other columns are zero.
    lconst = nc.dram_tensor("lconst", [2, BPC, P, 2, P], dt.float8e4,
                            kind="ExternalInput")
    y = nc.dram_tensor("y", [TOK, O_PER], dt.float32, kind="ExternalOutput")
    nc.tensors = {"xt": xt, "y": y}

    with tile.TileContext(nc) as tc:
        with (
            tc.tile_pool(name="wpool", bufs=1) as wpool,
            tc.tile_pool(name="cpool", bufs=1) as cpool,
            tc.tile_pool(name="mpool", bufs=6) as mpool,
            tc.tile_pool(name="xpool", bufs=3) as xpool,
            tc.tile_pool(name="ypool", bufs=3) as ypool,
            tc.tile_pool(name="dqps", bufs=2, space="PSUM") as dqps,
            tc.tile_pool(name="psum", bufs=4, space="PSUM") as psum,
        ):
            w = wpool.tile([P, IC, O_PER], dt.bfloat16)

            lc = cpool.tile([P, 2, BPC, 2, P], dt.float8e4, tag="lc")
            nc.sync.dma_start(lc[:], lconst[:].rearrange("s j ki ko p -> ki s j ko p"))

            # ---- Phase 1: dequantize W^T slice on the PE (exact) ----
            # fp8 DoubleRow: contraction 256 = (ki=128) x (ko=2) per matmul,
            # 2 fp8 MACs/cell/cycle -> each [32-row x 512] block in one MM.
            dr = mybir.MatmulPerfMode.DoubleRow
            pos_dmas = []
            for ic in range(IC):
                pos_g = mpool.tile([P, BPC, 2, O_PER], dt.float8e4, tag="pos")
                neg_g = mpool.tile([P, BPC, 2, O_PER], dt.float8e4, tag="neg")
                # pos on the SP ring, neg on the Activation ring: the two HWDGE
                # FIFOs deliver mask planes in parallel, halving delivery time.
                pos_dmas.append(nc.sync.dma_start(
                    pos_g[:], posm[ic * BPC:(ic + 1) * BPC]
                    .rearrange("b p ko o -> p b ko o")))
                nc.scalar.dma_start(
                    neg_g[:], negm[ic * BPC:(ic + 1) * BPC]
                    .rearrange("b p ko o -> p b ko o"))
                ps = dqps.tile([P, O_PER], dt.float32)
                for j in range(BPC):
                    nc.tensor.matmul(ps[:], lc[:, 0, j, :, :], pos_g[:, j, :, :],
                                     start=(j == 0), stop=False, perf_mode=dr)
                    nc.tensor.matmul(ps[:], lc[:, 1, j, :, :], neg_g[:, j, :, :],
                                     start=False, stop=(j == BPC - 1), perf_mode=dr)
                nc.any.tensor_copy(w[:, ic, :], ps[:])

            # ---- Phase 2: main matmul ----
            late = sorted({max(0, IC * 13 // 16), max(0, IC * 15 // 16), IC - 1})
            _build_common(nc, mybir, tile, tc, (xpool, ypool, psum), w, inv_scale,
                          late_mask_dmas=[pos_dmas[i] for i in late])

    nc.compile()
    return nc


def _build_program_packed(c0_scale):
    """bits form a ladder (bits[j] = bits[0]+j): planes bit-pack into one byte
    per weight on host; device computes W = Wp - Wn (exact in bf16) and folds
    2^bits[0]/scale into the output copy."""
    import concourse.mybir as mybir
    import concourse.tile as tile
    from concourse import bacc

    dt = mybir.dt
    nc = bacc.Bacc("TRN2", target_bir_lowering=False, debug=False)

    xt = nc.dram_tensor("xt", [D_IN, TOK], dt.bfloat16, kind="ExternalInput")
    wpos = nc.dram_tensor("wpos", [IC, P, O_PER], dt.uint8, kind="ExternalInput")
    wneg = nc.dram_tensor("wneg", [IC, P, O_PER], dt.uint8, kind="ExternalInput")
    y = nc.dram_tensor("y", [TOK, O_PER], dt.float32, kind="ExternalOutput")
    nc.tensors = {"xt": xt, "y": y}

    with tile.TileContext(nc) as tc:
        with (
            tc.tile_pool(name="wpool", bufs=1) as wpool,
            tc.tile_pool(name="mpool", bufs=1) as mpool,
            tc.tile_pool(name="xpool", bufs=3) as xpool,
            tc.tile_pool(name="ypool", bufs=3) as ypool,
            tc.tile_pool(name="psum", bufs=6, space="PSUM") as psum,
        ):
            w = wpool.tile([P, IC, O_PER], dt.bfloat16)
            wp = mpool.tile([P, IC, O_PER], dt.uint8, tag="wp")
            wn = mpool.tile([P, IC, O_PER], dt.uint8, tag="wn")
            # Packed masks ride the Activation HWDGE ring (x owns the SP ring
            # from t=0); quarter-DMAs interleave wp/wn so the first W chunks
            # are ready within a few microseconds.
            NQ = 4
            qc = IC // NQ
            for q in range(NQ):
                qs = slice(q * qc, (q + 1) * qc)
                nc.scalar.dma_start(wp[:, qs, :],
                                    wpos[qs].rearrange("ic p o -> p ic o"))
                nc.scalar.dma_start(wn[:, qs, :],
                                    wneg[qs].rearrange("ic p o -> p ic o"))
            for ic in range(IC):
                nc.vector.tensor_tensor(
                    w[:, ic, :], wp[:, ic, :], wn[:, ic, :],
                    mybir.AluOpType.subtract)

            _build_common(nc, mybir, tile, tc, (xpool, ypool, psum), w, c0_scale)

    nc.compile()
    return nc


def _build_program_dve(coeffs, inv_scale):
    import concourse.mybir as mybir
    import concourse.tile as tile
    from concourse import bacc

    dt = mybir.dt
    nc = bacc.Bacc("TRN2", target_bir_lowering=False, debug=False)

    xt = nc.dram_tensor("xt", [D_IN, TOK], dt.bfloat16, kind="ExternalInput")
    posm = nc.dram_tensor("posm", [IC, P, K, O_PER], dt.uint8, kind="ExternalInput")
    negm = nc.dram_tensor("negm", [IC, P, K, O_PER], dt.uint8, kind="ExternalInput")
    y = nc.dram_tensor("y", [TOK, O_PER], dt.float32, kind="ExternalOutput")
    nc.tensors = {"xt": xt, "y": y}

    with tile.TileContext(nc) as tc:
        with (
            tc.tile_pool(name="wpool", bufs=1) as wpool,
            tc.tile_pool(name="mpool", bufs=6) as mpool,
            tc.tile_pool(name="dpool", bufs=2) as dpool,
            tc.tile_pool(name="xpool", bufs=3) as xpool,
            tc.tile_pool(name="ypool", bufs=3) as ypool,
            tc.tile_pool(name="psum", bufs=4, space="PSUM") as psum,
        ):
            w = wpool.tile([P, IC, O_PER], dt.bfloat16)

            for ic in range(IC):
                pos8 = mpool.tile([P, K, O_PER], dt.uint8, tag="pos")
                neg8 = mpool.tile([P, K, O_PER], dt.uint8, tag="neg")
                nc.sync.dma_start(pos8[:], posm[ic])
                nc.sync.dma_start(neg8[:], negm[ic])
                acc = w[:, ic, :]
                for k in range(K):
                    if k == 0:
                        nc.vector.tensor_tensor(
                            acc, pos8[:, k, :], neg8[:, k, :],
                            mybir.AluOpType.subtract)
                        if coeffs[0] != 1.0:
                            nc.vector.tensor_scalar_mul(acc, acc, float(coeffs[0]))
                    else:
                        d = dpool.tile([P, O_PER], dt.bfloat16, tag="dig")
                        nc.vector.tensor_tensor(
                            d[:], pos8[:, k, :], neg8[:, k, :],
                            mybir.AluOpType.subtract)
                        nc.vector.tensor_scalar_mul(d[:], d[:], float(coeffs[k]))
                        nc.vector.tensor_add(acc, acc, d[:])

            _build_common(nc, mybir, tile, tc, (xpool, ypool, psum), w, inv_scale)

    nc.compile()
    return nc


def _fp8_exact(vals):
    f8 = ml_dtypes.float8_e4m3
    return all(float(f8(v)) == float(v) for v in vals)


def _stage_masks_pe(masks, sl):
    # DoubleRow rhs: [b32, ki=(k,i16), ko, o] where i_local = i16*2 + ko.
    NB32 = D_IN // (2 * IB)
    a = masks[:, sl, :].transpose(2, 0, 1)                 # [D_IN, K, O_PER]
    a = a.reshape(NB32, IB, 2, K, O_PER).transpose(0, 3, 1, 2, 4)
    return np.ascontiguousarray(a).reshape(NB32, P, 2, O_PER) \
        .astype(ml_dtypes.float8_e4m3)


def _stage_masks_dve(masks, sl):
    return masks[:, sl, :].transpose(2, 0, 1).astype(np.uint8).reshape(IC, P, K, O_PER)


def _stage_masks_packed(masks, sl):
    # Pure bit-packing: byte b[o, i] has bit j = plane j's boolean (packbits).
    a = np.ascontiguousarray(masks[:, sl, :])              # [K, O_PER, D_IN]
    b = np.packbits(a, axis=0, bitorder="little")[0]       # [O_PER, D_IN] u8
    return np.ascontiguousarray(b.T).reshape(IC, P, O_PER)


def kernel(x, pos_masks, neg_masks, bits, scale):
    global LAST_RESULTS
    from concourse.bass_utils import run_bass_kernel_spmd

    x = np.asarray(x)
    pos_masks = np.asarray(pos_masks)
    neg_masks = np.asarray(neg_masks)
    bits = np.asarray(bits)
    scale_f = float(np.asarray(scale))

    coeffs = np.exp2(bits.astype(np.float64))
    inv_scale = 1.0 / scale_f

    mode = DEQUANT_MODE
    bits_l = bits.astype(np.int64)
    is_ladder = K == 8 and bool(np.all(bits_l - bits_l[0] == np.arange(K)))
    if mode == "pe":
        if is_ladder:
            mode = "hybrid"
        elif not _fp8_exact(coeffs):
            mode = "dve"

    key = (mode, tuple(coeffs.tolist()), inv_scale)
    if key not in _CACHE:
        if mode == "hybrid":
            _CACHE[key] = _build_program_hybrid(float(coeffs[0] * inv_scale))
        elif mode == "packed":
            _CACHE[key] = _build_program_packed(float(coeffs[0] * inv_scale))
        elif mode == "pe":
            _CACHE[key] = _build_program_pe(coeffs, inv_scale)
        else:
            _CACHE[key] = _build_program_dve(coeffs, inv_scale)
    nc = _CACHE[key]

    if mode == "hybrid":
        # Host staging: x -> [D_IN, TOK] bf16, sliced into token halves;
        # masks packbits -> byte planes [IC, 128, O_C] per out-quarter.
        xt_full = x.reshape(TOK, D_IN).T.astype(ml_dtypes.bfloat16)
        wp_full = np.packbits(pos_masks, axis=0, bitorder="little")[0]  # [O, I]
        wn_full = np.packbits(neg_masks, axis=0, bitorder="little")[0]
        wp_t = wp_full.T   # [I, O]
        wn_t = wn_full.T
        xt_halves = [np.ascontiguousarray(xt_full[:, g * T_C:(g + 1) * T_C])
                     for g in range(G_TOK)]
        wm_q = []
        for h in range(H_OUT):
            wmq = np.empty((IC, 2, P, 2, 512), dtype=np.uint8)
            for half in range(2):
                o0 = h * O_C + half * 512
                wmq[:, half, :, 0, :] = \
                    wp_t[:, o0:o0 + 512].reshape(IC, P, 512)
                wmq[:, half, :, 1, :] = \
                    wn_t[:, o0:o0 + 512].reshape(IC, P, 512)
            wm_q.append(wmq)
        in_maps = []
        for c in range(N_CORES):
            g, h = c // H_OUT, c % H_OUT
            in_maps.append({"xt": xt_halves[g], "wm": wm_q[h]})
        res = run_bass_kernel_spmd(nc, in_maps, core_ids=list(range(N_CORES)))
        LAST_RESULTS = res
        yout = np.empty((TOK, D_OUT), dtype=np.float32)
        for c in range(N_CORES):
            g, h = c // H_OUT, c % H_OUT
            yout[g * T_C:(g + 1) * T_C, h * O_C:(h + 1) * O_C] = \
                res.results[c]["y"]
        return yout.reshape(B, T, D_OUT)

    # Host-side staging: transpose x to [D_IN, TOK] bf16 (shared by all cores).
    xt = x.reshape(TOK, D_IN).T.astype(ml_dtypes.bfloat16)

    if mode == "pe":
        f8 = ml_dtypes.float8_e4m3
        BPC = P // (2 * IB)
        lconst = np.zeros((2, BPC, P, 2, P), dtype=np.float32)
        for j in range(BPC):
            for k in range(K):
                for i16 in range(IB):
                    for ko in range(2):
                        p = j * 2 * IB + i16 * 2 + ko
                        lconst[0, j, k * IB + i16, ko, p] = coeffs[k]
                        lconst[1, j, k * IB + i16, ko, p] = -coeffs[k]
        lconst = lconst.astype(f8)

    in_maps = []
    for c in range(N_CORES):
        sl = slice(c * O_PER, (c + 1) * O_PER)
        if mode == "packed":
            in_maps.append({
                "xt": xt,
                "wpos": _stage_masks_packed(pos_masks, sl),
                "wneg": _stage_masks_packed(neg_masks, sl),
            })
        elif mode == "pe":
            in_maps.append({
                "xt": xt,
                "posm": _stage_masks_pe(pos_masks, sl),
                "negm": _stage_masks_pe(neg_masks, sl),
                "lconst": lconst,
            })
        else:
            in_maps.append({
                "xt": xt,
                "posm": _stage_masks_dve(pos_masks, sl),
                "negm": _stage_masks_dve(neg_masks, sl),
            })

    res = run_bass_kernel_spmd(nc, in_maps, core_ids=list(range(N_CORES)))
    LAST_RESULTS = res

    y = np.concatenate([res.results[c]["y"] for c in range(N_CORES)], axis=1)
    return np.ascontiguousarray(y.reshape(B, T, D_OUT).astype(np.float32))



# revision 24
# speedup vs baseline: 1.0105x; 1.0105x over previous
"""Trainium2 Bass kernel for nn_BinarySurrogateBlock.

Computes y = x @ W^T where W = (sum_k 2^bits[k] * (pos_k - neg_k)) / scale.

Primary mode "hybrid" (ladder bits, the spec's arange fill): 8 cores =
2 token-halves x 4 out-quarters; per core 8192 tokens x 1024 outs x full
4096 contraction. The bit planes pack into one byte per weight per sign
(packbits); W = wp - wn is dequantized once on the DVE (exact in bf16,
|W_int| <= 255) and kept SBUF-resident while x streams through as the
stationary matmul operand (bf16 x bf16 -> fp32 PSUM, N=512 moving). The
4096 matmuls/core run back-to-back at the 512-cycle PE floor; startup
streams masks + first x groups in ic-granular, ring-balanced chunks so
the PE crawls with arrival instead of stalling (the first ~90 us are
chip-HBM-bound), and y is written as fp16 (halved write traffic; error
is far below the bf16 input rounding that dominates).

Note: 8-bit PE modes cannot beat this. FP8 DoubleRow measures the same
216 ns per N=512 matmul as bf16 (2x MACs per matmul via 256-contraction,
so 157 TF/s peak), but exact W in [-255, 255] needs two e4m3 planes
(16*Whi + Wlo), which exactly cancels the 2x. Single-plane fp8 x fails
the 2e-2 gate (measured rel err 2.6e-2); uint8 matmul is rejected by
bass.

Fallback modes for non-ladder bits: "pe" (fp8 DoubleRow dequant against
+/-2^bits patterns) and "dve" (vector-engine plane accumulation), both
feeding the original tensor-parallel-over-d_out main loop.
"""

import numpy as np
import ml_dtypes

# Problem shape (hardcoded per contract; kernel.py must be self-contained).
B, T, D_IN, D_OUT, K = 8, 2048, 4096, 4096, 8
N_CORES = 8
TOK = B * T                    # 16384 tokens
O_PER = D_OUT // N_CORES       # 512 outputs per core
P = 128                        # partitions
IC = D_IN // P                 # 32 contraction chunks
TSUP = 512                     # token super-tile width
NSUP = TOK // TSUP             # 32 super-tiles
TS_PER = TSUP // P             # 4 psum tiles per super-tile
IB = 16                        # i-rows dequantized per PE-dequant matmul
NB = D_IN // IB                # 256 dequant blocks
BG = 4                         # blocks per mask DMA
DEQUANT_MODE = "pe"

LAST_RESULTS = None            # BassKernelResults of the last run (for test.py)

_CACHE = {}

# ---- Hybrid-sharded bf16 kernel (primary path) ----
# 8 cores = 2 token-halves x 4 out-quarters. Per core: 8192 tokens x 1024
# outs x full 4096 contraction. W = packbits(pos) - packbits(neg) (int,
# |W|<=255, exact in bf16) is dequantized once on the DVE and kept
# SBUF-resident; x streams through as the stationary matmul operand.
# Chip-wide HBM traffic drops from ~1.3 GB (TP-8) to ~0.83 GB, removing
# the DMA-contention stall mode, while the PE stays at the same
# 4096-matmul cycle floor.
G_TOK = 2                      # token-parallel ways
H_OUT = 4                      # out-parallel ways
T_C = TOK // G_TOK             # 8192 tokens per core
O_C = D_OUT // H_OUT           # 1024 outs per core
SW = 512                       # tokens per sweep (4 psum tchunks x 2 oc = 8 banks)
NSW = T_C // SW                # 16 sweeps
OC2 = O_C // 512               # 2 psum-width output chunks


def _build_program_hybrid(c0_scale):
    import concourse.mybir as mybir
    import concourse.tile as tile
    from concourse import bacc
    from concourse.tile_rust import add_dep_helper

    dt = mybir.dt
    nc = bacc.Bacc("TRN2", target_bir_lowering=False, debug=False)

    TG = 1024                  # tokens per group (8 psum banks x 128)
    NG = T_C // TG             # 8 groups per core

    xt = nc.dram_tensor("xt", [D_IN, T_C], dt.bfloat16, kind="ExternalInput")
    wm = nc.dram_tensor("wm", [IC, 2, P, 2, 512], dt.uint8,
                        kind="ExternalInput")
    y = nc.dram_tensor("y", [T_C, O_C], dt.float16, kind="ExternalOutput")
    nc.tensors = {"xt": xt, "y": y}

    with tile.TileContext(nc) as tc:
        with (
            tc.tile_pool(name="wres", bufs=1) as wres,
            tc.tile_pool(name="mpool", bufs=6) as mpool,
            tc.tile_pool(name="xpool", bufs=2) as xpool,
            tc.tile_pool(name="ypool", bufs=2) as ypool,
            tc.tile_pool(name="psum", bufs=8, space="PSUM") as psum,
        ):
            xt_v = xt.rearrange("(ic p) t -> p ic t", p=P)   # [128, IC, T_C]
            y_vp = y.rearrange("(n p) o -> p n o", p=P)      # [128, T_C//P, O_C]
            w = wres.tile([P, IC, O_C], dt.bfloat16)

            # Per-HWDGE-ring FIFO order: chain each DMA behind the previous
            # one on its ring so scheduler priorities can't reorder the
            # startup mask stream vs later prefetches.
            last_dma = {"sync": None, "act": None}

            def ring_dma(ring, dst, src):
                eng = nc.sync if ring == "sync" else nc.scalar
                dma = eng.dma_start(dst, src)
                if last_dma[ring] is not None:
                    add_dep_helper(dma.ins, last_dma[ring].ins, sync=False,
                                   reason="ring FIFO order")
                last_dma[ring] = dma
                return dma

            # Drain 8 psum banks of one sweep (g, oc): scale-copy each bank
            # to fp16 (alternating scalar/vector engines so bunched closes
            # pipeline), then write y as two 4-bank DMAs on the sync ring.
            def drain_sweep(banks, g, oc, fine=False):
                for half in range(2):
                    yt = ypool.tile([P, 4, 512], dt.float16, tag="yt")
                    for q in range(4):
                        b = half * 4 + q
                        if b % 2 == 0:
                            nc.scalar.activation(
                                yt[:, q, :], banks[b][:],
                                mybir.ActivationFunctionType.Copy,
                                scale=float(c0_scale))
                        else:
                            nc.vector.tensor_scalar_mul(
                                yt[:, q, :], banks[b][:], float(c0_scale))
                        if fine:
                            # Tail sweep: ship each bank as soon as its
                            # scale-copy lands so the final transfer is
                            # 128 KB, not 512 KB, and rides both rings.
                            n = g * 8 + b
                            ring_dma("sync" if b % 2 == 0 else "act",
                                     y_vp[:, n:n + 1,
                                          oc * 512:(oc + 1) * 512],
                                     yt[:, q:q + 1, :])
                    if not fine:
                        n0 = g * 8 + half * 4
                        ring_dma("sync",
                                 y_vp[:, n0:n0 + 4, oc * 512:(oc + 1) * 512],
                                 yt[:])

            def mm_sweep_ic(banks, xg, ic, oc, first, last):
                for tcn in range(8):
                    nc.tensor.matmul(
                        banks[tcn],
                        xg[:, ic, tcn * P:(tcn + 1) * P],
                        w[:, ic, oc * 512:(oc + 1) * 512],
                        start=first, stop=last)

            psA = [psum.tile([P, 512], dt.float32, name=f"psA_{b}", tag="ps")
                   for b in range(8)]

            # Sweep A: group 0, oc 0, ic-major. Per ic: lo-half masks on the
            # act ring, x split 3:1 sync:act, 8 matmuls. Startup is
            # chip-HBM-bound, so demand is balanced across rings and kept
            # fine-grained (sub-us stalls keep the HAM gate open).
            xg0 = xpool.tile([P, IC, TG], dt.bfloat16, tag="xs")
            b_pre = {}
            for ic in range(IC):
                wpn = mpool.tile([P, 2, 512], dt.uint8, tag="wpn")
                ring_dma("act" if ic % 2 == 0 else "sync", wpn[:], wm[ic, 0])
                nc.vector.tensor_tensor(w[:, ic, 0:512], wpn[:, 0, :],
                                        wpn[:, 1, :], mybir.AluOpType.subtract)
                ring_dma("sync" if ic % 2 == 0 else "act",
                         xg0[:, ic, :], xt_v[:, ic, 0:TG])
                mm_sweep_ic(psA, xg0, ic, 0, ic == 0, ic == IC - 1)
                if ic >= IC - 8 and (ic - (IC - 8)) % 2 == 0:
                    # Pre-stage sweep B's first mask chunks so B starts
                    # without waiting for the ring to drain A's stream.
                    j = (ic - (IC - 8)) // 2
                    wpn_b = mpool.tile([P, 2, 512], dt.uint8, tag="wpn")
                    ring_dma("act" if j % 2 == 0 else "sync",
                             wpn_b[:], wm[j, 1])
                    b_pre[j] = wpn_b
            drain_sweep(psA, 0, 0)

            # Sweep B: group 0, oc 1, ic-major. Hi-half masks on the act
            # ring; x is already resident. Group 1's x prefetches into the
            # spare sync bandwidth.
            xg1 = xpool.tile([P, IC, TG], dt.bfloat16, tag="xs")
            psB = [psum.tile([P, 512], dt.float32, name=f"psB_{b}", tag="ps")
                   for b in range(8)]
            for ic in range(IC):
                if ic in b_pre:
                    wpn = b_pre.pop(ic)
                else:
                    wpn = mpool.tile([P, 2, 512], dt.uint8, tag="wpn")
                    ring_dma("act" if ic % 2 == 0 else "sync",
                             wpn[:], wm[ic, 1])
                nc.vector.tensor_tensor(w[:, ic, 512:O_C], wpn[:, 0, :],
                                        wpn[:, 1, :], mybir.AluOpType.subtract)
                if ic < IC - 8:
                    ring_dma("sync" if ic % 2 == 0 else "act",
                             xg1[:, ic, :], xt_v[:, ic, TG:2 * TG])
                mm_sweep_ic(psB, xg0, ic, 1, ic == 0, ic == IC - 1)
            drain_sweep(psB, 0, 1)

            # Steady state: for each group, oc 0 then oc 1, tc-major so psum
            # banks close staggered. The next group's x rides the act ring
            # (idle after the masks), so y drains on sync are never blocked
            # behind a bulk transfer.
            # Group 1, oc 0: still draining the tail of xg1's transfer, so
            # run ic-major to crawl with arrival instead of a flat stall.
            psC = [psum.tile([P, 512], dt.float32, name=f"psC_{b}", tag="ps")
                   for b in range(8)]
            for ic in range(IC - 8, IC):
                ring_dma("sync" if ic % 2 == 0 else "act",
                         xg1[:, ic, :], xt_v[:, ic, TG:2 * TG])
            for ic in range(IC):
                mm_sweep_ic(psC, xg1, ic, 0, ic == 0, ic == IC - 1)
            drain_sweep(psC, 1, 0)

            xg_cur = xg1
            for g in range(1, NG):
                xg_nxt = None
                if g + 1 < NG:
                    xg_nxt = xpool.tile([P, IC, TG], dt.bfloat16, tag="xs")
                for oc in range(OC2):
                    if g == 1 and oc == 0:
                        if xg_nxt is not None:
                            ring_dma("act", xg_nxt[:, 0:IC // 2, :],
                                     xt_v[:, 0:IC // 2,
                                          (g + 1) * TG:(g + 2) * TG])
                        continue
                    if xg_nxt is not None:
                        ring_dma("act",
                                 xg_nxt[:, oc * (IC // 2):(oc + 1) * (IC // 2), :],
                                 xt_v[:, oc * (IC // 2):(oc + 1) * (IC // 2),
                                      (g + 1) * TG:(g + 2) * TG])
                    banks = []
                    for tcn in range(8):
                        ps = psum.tile([P, 512], dt.float32, tag="ps")
                        for ic in range(IC):
                            nc.tensor.matmul(
                                ps[:],
                                xg_cur[:, ic, tcn * P:(tcn + 1) * P],
                                w[:, ic, oc * 512:(oc + 1) * 512],
                                start=(ic == 0), stop=(ic == IC - 1))
                        banks.append(ps)
                    drain_sweep(banks, g, oc,
                                fine=(g == NG - 1 and oc == OC2 - 1))
                xg_cur = xg_nxt

    nc.compile()
    return nc


def _build_common(nc, mybir, tile, tc, pools, w, inv_scale, late_mask_dmas=None):
    """Main matmul phase: x-stationary, psum [128 tokens, 512 outs]."""
    from concourse.tile_rust import add_dep_helper
    dt = mybir.dt
    xpool, ypool, psum = pools
    xt = nc.tensors["xt"]
    y = nc.tensors["y"]
    xt_v = xt.rearrange("(ic p) t -> p ic t", p=P)     # [128, IC, TOK]
    y_v = y.rearrange("(n p) o -> n p o", p=P)         # [TOK//P, 128, O_PER]
    for st in range(NSUP):
        xt_t = xpool.tile([P, IC, TSUP], dt.bfloat16)
        # First super-tiles arrive in smaller pieces so the mains can start
        # as soon as the first token sub-tile lands (startup HBM congestion).
        npiece = 4 if st == 0 else (2 if st == 1 else 1)
        pw = TSUP // npiece
        for pc in range(npiece):
            x_dma = nc.sync.dma_start(
                xt_t[:, :, pc * pw:(pc + 1) * pw],
                xt_v[:, :, st * TSUP + pc * pw:st * TSUP + (pc + 1) * pw])
            if late_mask_dmas is not None and st < len(late_mask_dmas):
                # Keep the hoistable x prefetches from injecting into the
                # latency-critical mask stream on the same HWDGE FIFO ring.
                add_dep_helper(
                    x_dma.ins, late_mask_dmas[st].ins, sync=False,
                    reason="delay x prefetch behind dequant mask stream")
        for ts in range(TS_PER):
            ps = psum.tile([P, O_PER], dt.float32)
            for ic in range(IC):
                nc.tensor.matmul(
                    ps[:],
                    xt_t[:, ic, ts * P:(ts + 1) * P],
                    w[:, ic, :],
                    start=(ic == 0),
                    stop=(ic == IC - 1),
                )
            yt = ypool.tile([P, O_PER], dt.float32)
            nc.scalar.activation(
                yt[:], ps[:], mybir.ActivationFunctionType.Copy,
                scale=float(inv_scale))
            nc.scalar.dma_start(y_v[st * TS_PER + ts], yt[:])


def _build_program_pe(coeffs, inv_scale):
    import concourse.mybir as mybir
    import concourse.tile as tile
    from concourse import bacc

    dt = mybir.dt
    nc = bacc.Bacc("TRN2", target_bir_lowering=False, debug=False)
    nc.tensors = {}

    BPC = P // (2 * IB)  # dequant blocks (32 i-rows) per W chunk (4)

    xt = nc.dram_tensor("xt", [D_IN, TOK], dt.bfloat16, kind="ExternalInput")
    # DoubleRow rhs layout: [32-row block, ki=(k,i16), ko, o]
    NB32 = D_IN // (2 * IB)
    posm = nc.dram_tensor("posm", [NB32, P, 2, O_PER], dt.float8e4,
                          kind="ExternalInput")
    negm = nc.dram_tensor("negm", [NB32, P, 2, O_PER], dt.float8e4,
                          kind="ExternalInput")
    # lconst[s, j, ki, ko, p]: +/- 2^bits patterns; group j places dequant
    # block j at output partitions [j*32, (j+1)*32); other columns are zero.
    lconst = nc.dram_tensor("lconst", [2, BPC, P, 2, P], dt.float8e4,
                            kind="ExternalInput")
    y = nc.dram_tensor("y", [TOK, O_PER], dt.float32, kind="ExternalOutput")
    nc.tensors = {"xt": xt, "y": y}

    with tile.TileContext(nc) as tc:
        with (
            tc.tile_pool(name="wpool", bufs=1) as wpool,
            tc.tile_pool(name="cpool", bufs=1) as cpool,
            tc.tile_pool(name="mpool", bufs=6) as mpool,
            tc.tile_pool(name="xpool", bufs=3) as xpool,
            tc.tile_pool(name="ypool", bufs=3) as ypool,
            tc.tile_pool(name="dqps", bufs=2, space="PSUM") as dqps,
            tc.tile_pool(name="psum", bufs=4, space="PSUM") as psum,
        ):
            w = wpool.tile([P, IC, O_PER], dt.bfloat16)

            lc = cpool.tile([P, 2, BPC, 2, P], dt.float8e4, tag="lc")
            nc.sync.dma_start(lc[:], lconst[:].rearrange("s j ki ko p -> ki s j ko p"))

            # ---- Phase 1: dequantize W^T slice on the PE (exact) ----
            # fp8 DoubleRow: contraction 256 = (ki=128) x (ko=2) per matmul,
            # 2 fp8 MACs/cell/cycle -> each [32-row x 512] block in one MM.
            dr = mybir.MatmulPerfMode.DoubleRow
            pos_dmas = []
            for ic in range(IC):
                pos_g = mpool.tile([P, BPC, 2, O_PER], dt.float8e4, tag="pos")
                neg_g = mpool.tile([P, BPC, 2, O_PER], dt.float8e4, tag="neg")
                # pos on the SP ring, neg on the Activation ring: the two HWDGE
                # FIFOs deliver mask planes in parallel, halving delivery time.
                pos_dmas.append(nc.sync.dma_start(
                    pos_g[:], posm[ic * BPC:(ic + 1) * BPC]
                    .rearrange("b p ko o -> p b ko o")))
                nc.scalar.dma_start(
                    neg_g[:], negm[ic * BPC:(ic + 1) * BPC]
                    .rearrange("b p ko o -> p b ko o"))
                ps = dqps.tile([P, O_PER], dt.float32)
                for j in range(BPC):
                    nc.tensor.matmul(ps[:], lc[:, 0, j, :, :], pos_g[:, j, :, :],
                                     start=(j == 0), stop=False, perf_mode=dr)
                    nc.tensor.matmul(ps[:], lc[:, 1, j, :, :], neg_g[:, j, :, :],
                                     start=False, stop=(j == BPC - 1), perf_mode=dr)
                nc.any.tensor_copy(w[:, ic, :], ps[:])

            # ---- Phase 2: main matmul ----
            late = sorted({max(0, IC * 13 // 16), max(0, IC * 15 // 16), IC - 1})
            _build_common(nc, mybir, tile, tc, (xpool, ypool, psum), w, inv_scale,
                          late_mask_dmas=[pos_dmas[i] for i in late])

    nc.compile()
    return nc


def _build_program_packed(c0_scale):
    """bits form a ladder (bits[j] = bits[0]+j): planes bit-pack into one byte
    per weight on host; device computes W = Wp - Wn (exact in bf16) and folds
    2^bits[0]/scale into the output copy."""
    import concourse.mybir as mybir
    import concourse.tile as tile
    from concourse import bacc

    dt = mybir.dt
    nc = bacc.Bacc("TRN2", target_bir_lowering=False, debug=False)

    xt = nc.dram_tensor("xt", [D_IN, TOK], dt.bfloat16, kind="ExternalInput")
    wpos = nc.dram_tensor("wpos", [IC, P, O_PER], dt.uint8, kind="ExternalInput")
    wneg = nc.dram_tensor("wneg", [IC, P, O_PER], dt.uint8, kind="ExternalInput")
    y = nc.dram_tensor("y", [TOK, O_PER], dt.float32, kind="ExternalOutput")
    nc.tensors = {"xt": xt, "y": y}

    with tile.TileContext(nc) as tc:
        with (
            tc.tile_pool(name="wpool", bufs=1) as wpool,
            tc.tile_pool(name="mpool", bufs=1) as mpool,
            tc.tile_pool(name="xpool", bufs=3) as xpool,
            tc.tile_pool(name="ypool", bufs=3) as ypool,
            tc.tile_pool(name="psum", bufs=6, space="PSUM") as psum,
        ):
            w = wpool.tile([P, IC, O_PER], dt.bfloat16)
            wp = mpool.tile([P, IC, O_PER], dt.uint8, tag="wp")
            wn = mpool.tile([P, IC, O_PER], dt.uint8, tag="wn")
            # Packed masks ride the Activation HWDGE ring (x owns the SP ring
            # from t=0); quarter-DMAs interleave wp/wn so the first W chunks
            # are ready within a few microseconds.
            NQ = 4
            qc = IC // NQ
            for q in range(NQ):
                qs = slice(q * qc, (q + 1) * qc)
                nc.scalar.dma_start(wp[:, qs, :],
                                    wpos[qs].rearrange("ic p o -> p ic o"))
                nc.scalar.dma_start(wn[:, qs, :],
                                    wneg[qs].rearrange("ic p o -> p ic o"))
            for ic in range(IC):
                nc.vector.tensor_tensor(
                    w[:, ic, :], wp[:, ic, :], wn[:, ic, :],
                    mybir.AluOpType.subtract)

            _build_common(nc, mybir, tile, tc, (xpool, ypool, psum), w, c0_scale)

    nc.compile()
    return nc


def _build_program_dve(coeffs, inv_scale):
    import concourse.mybir as mybir
    import concourse.tile as tile
    from concourse import bacc

    dt = mybir.dt
    nc = bacc.Bacc("TRN2", target_bir_lowering=False, debug=False)

    xt = nc.dram_tensor("xt", [D_IN, TOK], dt.bfloat16, kind="ExternalInput")
    posm = nc.dram_tensor("posm", [IC, P, K, O_PER], dt.uint8, kind="ExternalInput")
    negm = nc.dram_tensor("negm", [IC, P, K, O_PER], dt.uint8, kind="ExternalInput")
    y = nc.dram_tensor("y", [TOK, O_PER], dt.float32, kind="ExternalOutput")
    nc.tensors = {"xt": xt, "y": y}

    with tile.TileContext(nc) as tc:
        with (
            tc.tile_pool(name="wpool", bufs=1) as wpool,
            tc.tile_pool(name="mpool", bufs=6) as mpool,
            tc.tile_pool(name="dpool", bufs=2) as dpool,
            tc.tile_pool(name="xpool", bufs=3) as xpool,
            tc.tile_pool(name="ypool", bufs=3) as ypool,
            tc.tile_pool(name="psum", bufs=4, space="PSUM") as psum,
        ):
            w = wpool.tile([P, IC, O_PER], dt.bfloat16)

            for ic in range(IC):
                pos8 = mpool.tile([P, K, O_PER], dt.uint8, tag="pos")
                neg8 = mpool.tile([P, K, O_PER], dt.uint8, tag="neg")
                nc.sync.dma_start(pos8[:], posm[ic])
                nc.sync.dma_start(neg8[:], negm[ic])
                acc = w[:, ic, :]
                for k in range(K):
                    if k == 0:
                        nc.vector.tensor_tensor(
                            acc, pos8[:, k, :], neg8[:, k, :],
                            mybir.AluOpType.subtract)
                        if coeffs[0] != 1.0:
                            nc.vector.tensor_scalar_mul(acc, acc, float(coeffs[0]))
                    else:
                        d = dpool.tile([P, O_PER], dt.bfloat16, tag="dig")
                        nc.vector.tensor_tensor(
                            d[:], pos8[:, k, :], neg8[:, k, :],
                            mybir.AluOpType.subtract)
                        nc.vector.tensor_scalar_mul(d[:], d[:], float(coeffs[k]))
                        nc.vector.tensor_add(acc, acc, d[:])

            _build_common(nc, mybir, tile, tc, (xpool, ypool, psum), w, inv_scale)

    nc.compile()
    return nc


def _fp8_exact(vals):
    f8 = ml_dtypes.float8_e4m3
    return all(float(f8(v)) == float(v) for v in vals)


def _stage_masks_pe(masks, sl):
    # DoubleRow rhs: [b32, ki=(k,i16), ko, o] where i_local = i16*2 + ko.
    NB32 = D_IN // (2 * IB)
    a = masks[:, sl, :].transpose(2, 0, 1)                 # [D_IN, K, O_PER]
    a = a.reshape(NB32, IB, 2, K, O_PER).transpose(0, 3, 1, 2, 4)
    return np.ascontiguousarray(a).reshape(NB32, P, 2, O_PER) \
        .astype(ml_dtypes.float8_e4m3)


def _stage_masks_dve(masks, sl):
    return masks[:, sl, :].transpose(2, 0, 1).astype(np.uint8).reshape(IC, P, K, O_PER)


def _stage_masks_packed(masks, sl):
    # Pure bit-packing: byte b[o, i] has bit j = plane j's boolean (packbits).
    a = np.ascontiguousarray(masks[:, sl, :])              # [K, O_PER, D_IN]
    b = np.packbits(a, axis=0, bitorder="little")[0]       # [O_PER, D_IN] u8
    return np.ascontiguousarray(b.T).reshape(IC, P, O_PER)


def kernel(x, pos_masks, neg_masks, bits, scale):
    global LAST_RESULTS
    from concourse.bass_utils import run_bass_kernel_spmd

    x = np.asarray(x)
    pos_masks = np.asarray(pos_masks)
    neg_masks = np.asarray(neg_masks)
    bits = np.asarray(bits)
    scale_f = float(np.asarray(scale))

    coeffs = np.exp2(bits.astype(np.float64))
    inv_scale = 1.0 / scale_f

    mode = DEQUANT_MODE
    bits_l = bits.astype(np.int64)
    is_ladder = K == 8 and bool(np.all(bits_l - bits_l[0] == np.arange(K)))
    if mode == "pe":
        if is_ladder:
            mode = "hybrid"
        elif not _fp8_exact(coeffs):
            mode = "dve"

    key = (mode, tuple(coeffs.tolist()), inv_scale)
    if key not in _CACHE:
        if mode == "hybrid":
            _CACHE[key] = _build_program_hybrid(float(coeffs[0] * inv_scale))
        elif mode == "packed":
            _CACHE[key] = _build_program_packed(float(coeffs[0] * inv_scale))
        elif mode == "pe":
            _CACHE[key] = _build_program_pe(coeffs, inv_scale)
        else:
            _CACHE[key] = _build_program_dve(coeffs, inv_scale)
    nc = _CACHE[key]

    if mode == "hybrid":
        # Host staging: x -> [D_IN, TOK] bf16, sliced into token halves;
        # masks packbits -> byte planes [IC, 128, O_C] per out-quarter.
        xt_full = x.reshape(TOK, D_IN).T.astype(ml_dtypes.bfloat16)
        wp_full = np.packbits(pos_masks, axis=0, bitorder="little")[0]  # [O, I]
        wn_full = np.packbits(neg_masks, axis=0, bitorder="little")[0]
        wp_t = wp_full.T   # [I, O]
        wn_t = wn_full.T
        xt_halves = [np.ascontiguousarray(xt_full[:, g * T_C:(g + 1) * T_C])
                     for g in range(G_TOK)]
        wm_q = []
        for h in range(H_OUT):
            wmq = np.empty((IC, 2, P, 2, 512), dtype=np.uint8)
            for half in range(2):
                o0 = h * O_C + half * 512
                wmq[:, half, :, 0, :] = \
                    wp_t[:, o0:o0 + 512].reshape(IC, P, 512)
                wmq[:, half, :, 1, :] = \
                    wn_t[:, o0:o0 + 512].reshape(IC, P, 512)
            wm_q.append(wmq)
        in_maps = []
        for c in range(N_CORES):
            g, h = c // H_OUT, c % H_OUT
            in_maps.append({"xt": xt_halves[g], "wm": wm_q[h]})
        res = run_bass_kernel_spmd(nc, in_maps, core_ids=list(range(N_CORES)))
        LAST_RESULTS = res
        yout = np.empty((TOK, D_OUT), dtype=np.float32)
        for c in range(N_CORES):
            g, h = c // H_OUT, c % H_OUT
            yout[g * T_C:(g + 1) * T_C, h * O_C:(h + 1) * O_C] = \
                res.results[c]["y"]
        return yout.reshape(B, T, D_OUT)

    # Host-side staging: transpose x to [D_IN, TOK] bf16 (shared by all cores).
    xt = x.reshape(TOK, D_IN).T.astype(ml_dtypes.bfloat16)

    if mode == "pe":
        f8 = ml_dtypes.float8_e4m3
        BPC = P // (2 * IB)
        lconst = np.zeros((2, BPC, P, 2, P), dtype=np.float32)
        for j in range(BPC):
            for k in range(K):
                for i16 in range(IB):
                    for ko in range(2):
                        p = j * 2 * IB + i16 * 2 + ko
                        lconst[0, j, k * IB + i16, ko, p] = coeffs[k]
                        lconst[1, j, k * IB + i16, ko, p] = -coeffs[k]
        lconst = lconst.astype(f8)

    in_maps = []
    for c in range(N_CORES):
        sl = slice(c * O_PER, (c + 1) * O_PER)
        if mode == "packed":
            in_maps.append({
                "xt": xt,
                "wpos": _stage_masks_packed(pos_masks, sl),
                "wneg": _stage_masks_packed(neg_masks, sl),
            })
        elif mode == "pe":
            in_maps.append({
                "xt": xt,
                "posm": _stage_masks_pe(pos_masks, sl),
                "negm": _stage_masks_pe(neg_masks, sl),
                "lconst": lconst,
            })
        else:
            in_maps.append({
                "xt": xt,
                "posm": _stage_masks_dve(pos_masks, sl),
                "negm": _stage_masks_dve(neg_masks, sl),
            })

    res = run_bass_kernel_spmd(nc, in_maps, core_ids=list(range(N_CORES)))
    LAST_RESULTS = res

    y = np.concatenate([res.results[c]["y"] for c in range(N_CORES)], axis=1)
    return np.ascontiguousarray(y.reshape(B, T, D_OUT).astype(np.float32))

